# revision 1
# baseline (speedup 1.0000x reference)
"""BitLinear forward on 8 TRN2 NeuronCores (tensor-parallel, column-parallel linear).

  alpha = mean(|W|)            (scalar over the FULL weight matrix)
  y     = x @ (sign(W) * alpha)^T

Sharding: W rows (out_features) split across 8 cores; x replicated; each core
computes y[:, c*2048:(c+1)*2048]. alpha is a scalar reduction over the local
shard on each core, combined across shards between the two launches (summing 8
partial scalars; the device does all O(n) work).

Two SPMD launches (found experimentally: a real 8-rank collective_compute in
the NEFF permanently downclocks the PE from 2.4GHz to ~2.0GHz for the whole
run, costing ~22% on every matmul — so the cross-core scalar reduction is NOT
done with a collective):

  Kernel A (prep, ~0.15ms): per core, load W shard fp32, sign() -> bf16,
    PE-transpose into K-major layout, cast fp8e4 (+-1 exact) -> output
    wt [128, 32, 2048]; also |W| row-sums -> partition_all_reduce -> scalar
    partial sum output.
  Kernel B (main, ~1.85ms): load wt into SBUF once; broadcast alpha; per
    128-row x tile: load fp32 -> cast bf16 -> SBUF->SBUF XBAR DMA-transpose ->
    xT [128, 32, 128]; 32x4 matmuls accumulate [128, 2048] fp32 in PSUM;
    ScalarE Copy*alpha eviction; DMA out.

Matmul mapping: out[s, o] += xT[i, s].T @ WT[i, o]  (K=i on partitions).
Mixed bf16(stationary) x fp8(moving) matmul runs at full bf16 rate.

Known pitfalls (verified on HW): XBAR transposes must all issue from nc.sync
(issuing some from nc.scalar corrupts data); removing "redundant" per-matmul
LDWEIGHTS corrupts results (PE weight-buffer management assumes self-loading);
a real multi-rank collective_compute downclocks the PE for the entire NEFF.
"""
import sys
import os

sys.path.insert(0, "/opt/trn_rl_repo")
import numpy as np

P = 128
S, I, O = 8192, 4096, 16384
N_CORES = 8
OC = O // N_CORES          # 2048 out-features per core
KB = I // P                # 32 contraction blocks
NT = S // P                # 64 x row-tiles
NJ = OC // 512             # 4 psum bank chunks

_cache = {}


def _build_prep():
    from concourse import bacc, tile, mybir, bass_isa
    from concourse.masks import make_identity

    dt = mybir.dt
    nc = bacc.Bacc("TRN2", target_bir_lowering=False, debug=False, num_devices=N_CORES)
    w_ap = nc.dram_tensor("w", [OC, I], dt.float32, kind="ExternalInput").ap()
    wt_ap = nc.dram_tensor("wt", [P, KB, OC], dt.float8e4, kind="ExternalOutput").ap()
    as_ap = nc.dram_tensor("asum", [1, 1], dt.float32, kind="ExternalOutput").ap()

    HI = I // 2
    HB = KB // 2

    with tile.TileContext(nc) as tc:
        with (
            tc.tile_pool(name="pers", bufs=1) as pers,
            tc.tile_pool(name="wld", bufs=8) as wld,
            tc.tile_pool(name="wsg", bufs=4) as wsg,
            tc.tile_pool(name="psum", bufs=4, space="PSUM") as psum,
        ):
            ident = pers.tile([P, P], dt.bfloat16)
            make_identity(nc, ident)
            WT = pers.tile([P, KB, OC], dt.float8e4)
            wabs = pers.tile([P, 2 * (OC // P)], dt.float32)
            for h in range(2):
                for t in range(OC // P):
                    w32 = wld.tile([P, HI], dt.float32, tag="wld")
                    nc.sync.dma_start(w32[:], w_ap[t * P:(t + 1) * P, h * HI:(h + 1) * HI])
                    sg = wsg.tile([P, HI], dt.bfloat16, tag="wsg")
                    nc.scalar.sign(sg[:], w32[:])
                    nc.vector.tensor_reduce(
                        wabs[:, 2 * t + h:2 * t + h + 1], w32[:],
                        axis=mybir.AxisListType.XYZW,
                        op=mybir.AluOpType.add, apply_absolute_value=True)
                    psT = psum.tile([P, HB, P], dt.bfloat16, tag="ps")
                    for b in range(HB):
                        nc.tensor.transpose(psT[:, b, :], sg[:, b * P:(b + 1) * P], ident[:])
                    wt_dst = WT[:, h * HB:(h + 1) * HB, t * P:(t + 1) * P]
                    if t % 2 == 0:
                        nc.scalar.activation(wt_dst, psT[:],
                                             mybir.ActivationFunctionType.Copy)
                    else:
                        nc.vector.tensor_copy(wt_dst, psT[:])
                # each half is contiguous in DRAM; storing per-half overlaps
                # the store with the second half's compute
                nc.sync.dma_start(wt_ap[:, h * HB:(h + 1) * HB, :],
                                  WT[:, h * HB:(h + 1) * HB, :])
            wsum = pers.tile([P, 1], dt.float32)
            nc.vector.tensor_reduce(
                wsum[:], wabs[:], axis=mybir.AxisListType.XYZW,
                op=mybir.AluOpType.add)
            par = pers.tile([P, 1], dt.float32)
            nc.gpsimd.partition_all_reduce(
                par[:], wsum[:], channels=P, reduce_op=bass_isa.ReduceOp.add)
            nc.sync.dma_start(as_ap, par[0:1, :])

    nc.compile()
    return nc


def _build_main():
    from concourse import bacc, tile, mybir

    dt = mybir.dt
    nc = bacc.Bacc("TRN2", target_bir_lowering=False, debug=False, num_devices=N_CORES)
    x_ap = nc.dram_tensor("x", [S, I], dt.float32, kind="ExternalInput").ap()
    wt_ap = nc.dram_tensor("wt", [P, KB, OC], dt.float8e4, kind="ExternalInput").ap()
    al_ap = nc.dram_tensor("al", [1, 1], dt.float32, kind="ExternalInput").ap()
    y_ap = nc.dram_tensor("y", [S, OC], dt.float32, kind="ExternalOutput").ap()

    with tile.TileContext(nc) as tc:
        with (
            tc.tile_pool(name="pers", bufs=1) as pers,
            tc.tile_pool(name="xld", bufs=3) as xld,
            tc.tile_pool(name="xsg", bufs=3) as xsg,
            tc.tile_pool(name="pxT", bufs=4) as pxT,
            tc.tile_pool(name="pyo", bufs=3) as pyo,
            tc.tile_pool(name="psum", bufs=2, space="PSUM") as psum,
        ):
            # fully prepare the first x tiles (incl. their XBAR transposes)
            # BEFORE the bulk WT load: a DMA-transpose serializes against all
            # in-flight plain DMAs (xbar mode switch), so issuing xT0 after the
            # 8MB WT load would stall it ~20us
            NPRE = 2
            preT = []
            for st in range(NPRE):
                x32 = xld.tile([P, I], dt.float32, tag="xld")
                nc.sync.dma_start(x32[:], x_ap[st * P:(st + 1) * P, :])
                xc = xsg.tile([P, I], dt.bfloat16, tag="xsg")
                nc.vector.tensor_copy(xc[:], x32[:])
                xT = pxT.tile([P, KB, P], dt.bfloat16, tag="xT")
                nc.sync.dma_start_transpose(xT[:], xc[:])
                preT.append(xT)
            a1 = pers.tile([1, 1], dt.float32)
            nc.sync.dma_start(a1[:], al_ap)
            WT = pers.tile([P, KB, OC], dt.float8e4)
            for c in range(8):
                # chunked so the first matmuls only wait for the k=0..3 piece
                nc.sync.dma_start(WT[:, 4 * c:4 * (c + 1), :], wt_ap[:, 4 * c:4 * (c + 1), :])
            ab = pers.tile([P, 1], dt.float32)
            nc.gpsimd.partition_broadcast(ab[:], a1[:])
            alpha = pers.tile([P, 1], dt.float32)
            nc.vector.tensor_scalar_mul(alpha[:], ab[:], 1.0 / (float(O) * float(I)))

            for st in range(NT):
                if st < NPRE:
                    xT = preT[st]
                else:
                    x32 = xld.tile([P, I], dt.float32, tag="xld")
                    nc.sync.dma_start(x32[:], x_ap[st * P:(st + 1) * P, :])
                    xc = xsg.tile([P, I], dt.bfloat16, tag="xsg")
                    nc.vector.tensor_copy(xc[:], x32[:])
                    xT = pxT.tile([P, KB, P], dt.bfloat16, tag="xT")
                    nc.sync.dma_start_transpose(xT[:], xc[:])
                ps = psum.tile([P, OC], dt.float32, tag="ps")
                for k in range(KB):
                    for j in range(NJ):
                        nc.tensor.matmul(
                            ps[:, j * 512:(j + 1) * 512],
                            xT[:, k, :],
                            WT[:, k, j * 512:(j + 1) * 512],
                            start=(k == 0), stop=(k == KB - 1))
                yo = pyo.tile([P, OC], dt.float32, tag="yo")
                nc.scalar.activation(
                    yo[:], ps[:], mybir.ActivationFunctionType.Copy,
                    bias=0.0, scale=alpha[:, 0:1])
                nc.sync.dma_start(y_ap[st * P:(st + 1) * P, :], yo[:])

    nc.compile()
    return nc


def _get_ncs():
    if "nc_main" not in _cache:
        _cache["nc_prep"] = _build_prep()
        _cache["nc_main"] = _build_main()
    return _cache["nc_prep"], _cache["nc_main"]


def kernel(x: np.ndarray, weight: np.ndarray) -> np.ndarray:
    from concourse.bass_utils import run_bass_kernel_spmd

    nc_prep, nc_main = _get_ncs()
    trace = bool(int(os.environ.get("BITLINEAR_TRACE", "0")))

    wf = np.asarray(weight, dtype=np.float32)
    in_a = [{"w": np.ascontiguousarray(wf[c * OC:(c + 1) * OC])} for c in range(N_CORES)]
    res_a = run_bass_kernel_spmd(nc_prep, in_a, core_ids=list(range(N_CORES)), trace=trace)

    total = np.float32(sum(res_a.results[c]["asum"][0, 0] for c in range(N_CORES)))
    al = np.array([[total]], dtype=np.float32)

    xf = np.ascontiguousarray(np.asarray(x, dtype=np.float32).reshape(S, I))
    in_b = [
        {"x": xf, "wt": res_a.results[c]["wt"], "al": al}
        for c in range(N_CORES)
    ]
    res_b = run_bass_kernel_spmd(nc_main, in_b, core_ids=list(range(N_CORES)), trace=trace)

    _cache["exec_time_ns_prep"] = res_a.exec_time_ns
    _cache["exec_time_ns_main"] = res_b.exec_time_ns
    if res_a.exec_time_ns is not None and res_b.exec_time_ns is not None:
        _cache["exec_time_ns"] = res_a.exec_time_ns + res_b.exec_time_ns
    y = np.concatenate([res_b.results[c]["y"] for c in range(N_CORES)], axis=1)
    return y.reshape(2, S // 2, O)

